# revision 1
# baseline (speedup 1.0000x reference)
"""Trainium2 Bass kernel for 2-layer LSTM + 3 dense heads + final linear.

Problem: x[128,1024,512] -> LSTM(512) -> LSTM(512) -> 3x Dense(8) concat -> Dense(24).

Sharding: data-parallel over batch across 8 cores (16 sequences/core);
weights replicated.  Head stack is folded host-side into one [512,24] matmul.

Per-core plan (all (t,b)-major so each time step's batch rows are contiguous):
  phase 1: xp1 = x @ W1 + b1       (batched matmul, PE-transpose x tiles for lhsT)
  phase 2: scan layer 1 over t:  z = xp1[t] + h @ U1 (PSUM accumulation with an
           identity-matmul adding xp1[t]), gates [i|f|o|g], cell update,
           h transposed back via PE for the next step's lhsT
  phase 3: xp2 = h1 @ W2 + b2
  phase 4: scan layer 2
  phase 5: out = h2 @ Wh_fold + bh_fold
"""

import numpy as np
from contextlib import ExitStack

import concourse.bass as bass
import concourse.tile as tile
from concourse import bacc, mybir
from concourse.bass import ds, ts
from concourse.bass_utils import run_bass_kernel_spmd
from concourse.masks import make_identity

F32 = mybir.dt.float32
AF = mybir.ActivationFunctionType

B, T, F, H, OUT = 128, 1024, 512, 512, 24
NCORES = 8
BS = B // NCORES        # 16 sequences per core
G = 4 * H               # 2048 gate width
R = T * BS              # rows per core, (t, b)-major
KC = H // 128           # 4 contraction chunks
NB = G // 512           # 4 gate n-blocks of 512


def _reorder_gates(w):
    """reference gate order [i f g o] -> kernel order [i f o g] (last axis)."""
    i, f, g, o = np.split(w, 4, axis=-1)
    return np.ascontiguousarray(np.concatenate([i, f, o, g], axis=-1))


def _xp_phase(nc, tc, ctx, src_flat, w_sb, b_sb, dst_flat, ident, ones, fin):
    """dst[R, G] = src[R, fin] @ W + b.  src rows processed in 128-row tiles."""
    xpool = ctx.enter_context(tc.tile_pool(name="xp_x", bufs=3))
    tpool = ctx.enter_context(tc.tile_pool(name="xp_xT", bufs=2))
    opool = ctx.enter_context(tc.tile_pool(name="xp_out", bufs=4))
    pt_pool = ctx.enter_context(tc.tile_pool(name="xp_pT", bufs=2, space="PSUM"))
    pm_pool = ctx.enter_context(tc.tile_pool(name="xp_pm", bufs=4, space="PSUM"))
    kc_in = fin // 128

    for m in range(R // 128):
        xt = xpool.tile([128, fin], F32)
        nc.sync.dma_start(out=xt[:], in_=src_flat[m * 128:(m + 1) * 128, :])
        # transpose input tile -> lhsT chunks [128, 128] each
        pT = pt_pool.tile([128, fin], F32)
        for k in range(kc_in):
            nc.tensor.transpose(
                pT[:, 128 * k:128 * (k + 1)],
                xt[:, 128 * k:128 * (k + 1)],
                ident[0:128, 0:128],
            )
        xTs = tpool.tile([128, fin], F32)
        nc.vector.tensor_copy(xTs[:], pT[:])
        for nb in range(G // 512):
            pm = pm_pool.tile([128, 512], F32)
            # bias via ones-row matmul: ones[1,128].T @ b[1,512]
            nc.tensor.matmul(
                pm[:], ones[0:1, 0:128], b_sb[0:1, 512 * nb:512 * (nb + 1)],
                start=True, stop=False,
            )
            for k in range(kc_in):
                nc.tensor.matmul(
                    pm[:],
                    xTs[:, 128 * k:128 * (k + 1)],
                    w_sb[k][:, 512 * nb:512 * (nb + 1)],
                    start=False, stop=(k == kc_in - 1),
                )
            ot = opool.tile([128, 512], F32)
            nc.scalar.copy(ot[:], pm[:])
            nc.sync.dma_start(
                out=dst_flat[m * 128:(m + 1) * 128, 512 * nb:512 * (nb + 1)],
                in_=ot[:],
            )


def _scan_phase(nc, tc, ctx, xp_flat, u_sb, h_dst_flat, ident, tag):
    """LSTM scan over T steps.  xp_flat[R, G] precomputed gate inputs,
    u_sb = 4 SBUF chunks of U [128, G], h written to h_dst_flat[R, H]."""
    state = ctx.enter_context(tc.tile_pool(name=f"st_{tag}", bufs=1))
    xpp = ctx.enter_context(tc.tile_pool(name=f"xps_{tag}", bufs=4))
    gp = ctx.enter_context(tc.tile_pool(name=f"gates_{tag}", bufs=2))
    hp = ctx.enter_context(tc.tile_pool(name=f"h_{tag}", bufs=2))
    zp = ctx.enter_context(tc.tile_pool(name=f"z_{tag}", bufs=1, space="PSUM"))
    ptp = ctx.enter_context(tc.tile_pool(name=f"pT_{tag}", bufs=2, space="PSUM"))

    c_t = state.tile([BS, H], F32, tag="c")
    hT = state.tile([128, KC * BS], F32, tag="hT")   # transposed h, chunk j at cols [16j,16j+16)
    nc.vector.memset(c_t[:], 0.0)
    nc.vector.memset(hT[:], 0.0)

    def body(t):
        xps = xpp.tile([BS, G], F32)
        nc.sync.dma_start(out=xps[:], in_=xp_flat[ds(t * BS, BS), :])
        z = zp.tile([BS, G], F32)
        for nb in range(NB):
            sl = slice(512 * nb, 512 * (nb + 1))
            # xp[t] added into PSUM via identity matmul, then accumulate h @ U
            nc.tensor.matmul(z[:, sl], ident[0:BS, 0:BS], xps[:, sl],
                             start=True, stop=False)
            for k in range(KC):
                nc.tensor.matmul(
                    z[:, sl],
                    hT[:, BS * k:BS * (k + 1)],
                    u_sb[k][:, sl],
                    start=False, stop=(k == KC - 1),
                )
        # gates: [i f o] sigmoid in one shot, g tanh
        sig = gp.tile([BS, 3 * H], F32, tag="sig")
        nc.scalar.activation(sig[:], z[:, 0:3 * H], AF.Sigmoid)
        tg = gp.tile([BS, H], F32, tag="tg")
        nc.scalar.activation(tg[:], z[:, 3 * H:4 * H], AF.Tanh)
        m1 = gp.tile([BS, H], F32, tag="m1")
        nc.vector.tensor_mul(m1[:], sig[:, 0:H], tg[:])
        m2 = gp.tile([BS, H], F32, tag="m2")
        nc.vector.tensor_mul(m2[:], sig[:, H:2 * H], c_t[:])
        nc.vector.tensor_add(c_t[:], m1[:], m2[:])
        tc_ = gp.tile([BS, H], F32, tag="tc")
        nc.scalar.activation(tc_[:], c_t[:], AF.Tanh)
        h = hp.tile([BS, H], F32)
        nc.vector.tensor_mul(h[:], sig[:, 2 * H:3 * H], tc_[:])
        nc.sync.dma_start(out=h_dst_flat[ds(t * BS, BS), :], in_=h[:])
        # transpose h for next step's lhsT
        pT = ptp.tile([128, KC * BS], F32)
        for j in range(KC):
            nc.tensor.transpose(
                pT[:, BS * j:BS * (j + 1)],
                h[:, 128 * j:128 * (j + 1)],
                ident[0:BS, 0:BS],
            )
        nc.vector.tensor_copy(hT[:], pT[:])

    tc.For_i_unrolled(0, T, 1, body, max_unroll=8)


def _heads_phase(nc, tc, ctx, h_flat, wh_sb, bh_sb, out_flat, ident, ones):
    """out[R, OUT] = h[R, H] @ Wh + bh."""
    xpool = ctx.enter_context(tc.tile_pool(name="hd_x", bufs=3))
    tpool = ctx.enter_context(tc.tile_pool(name="hd_xT", bufs=2))
    opool = ctx.enter_context(tc.tile_pool(name="hd_out", bufs=4))
    ptp = ctx.enter_context(tc.tile_pool(name="hd_pT", bufs=2, space="PSUM"))
    pop = ctx.enter_context(tc.tile_pool(name="hd_po", bufs=4, space="PSUM"))

    for m in range(R // 128):
        ht = xpool.tile([128, H], F32)
        nc.sync.dma_start(out=ht[:], in_=h_flat[m * 128:(m + 1) * 128, :])
        pT = ptp.tile([128, H], F32)
        for k in range(KC):
            nc.tensor.transpose(
                pT[:, 128 * k:128 * (k + 1)],
                ht[:, 128 * k:128 * (k + 1)],
                ident[0:128, 0:128],
            )
        hTs = tpool.tile([128, H], F32)
        nc.vector.tensor_copy(hTs[:], pT[:])
        po = pop.tile([128, OUT], F32)
        nc.tensor.matmul(po[:], ones[0:1, 0:128], bh_sb[0:1, :],
                         start=True, stop=False)
        for k in range(KC):
            nc.tensor.matmul(po[:], hTs[:, 128 * k:128 * (k + 1)], wh_sb[k][:],
                             start=False, stop=(k == KC - 1))
        ot = opool.tile([128, OUT], F32)
        nc.scalar.copy(ot[:], po[:])
        nc.sync.dma_start(out=out_flat[m * 128:(m + 1) * 128, :], in_=ot[:])


def _build():
    nc = bacc.Bacc("TRN2", target_bir_lowering=False, debug=False,
                   enable_asserts=False, num_devices=NCORES)
    xin = nc.dram_tensor("xin", [R, F], F32, kind="ExternalInput")
    w1 = nc.dram_tensor("w1", [F, G], F32, kind="ExternalInput")
    u1 = nc.dram_tensor("u1", [H, G], F32, kind="ExternalInput")
    b1 = nc.dram_tensor("b1", [1, G], F32, kind="ExternalInput")
    w2 = nc.dram_tensor("w2", [H, G], F32, kind="ExternalInput")
    u2 = nc.dram_tensor("u2", [H, G], F32, kind="ExternalInput")
    b2 = nc.dram_tensor("b2", [1, G], F32, kind="ExternalInput")
    wh = nc.dram_tensor("wh", [H, OUT], F32, kind="ExternalInput")
    bh = nc.dram_tensor("bh", [1, OUT], F32, kind="ExternalInput")
    out = nc.dram_tensor("out", [R, OUT], F32, kind="ExternalOutput")

    with tile.TileContext(nc) as tc, ExitStack() as top:
        dram = top.enter_context(tc.tile_pool(name="dram", bufs=1, space="DRAM"))
        xp_d = dram.tile([R, G], F32, tag="xp1")
        xp2_d = dram.tile([R, G], F32, tag="xp2")
        h1_d = dram.tile([R, H], F32, tag="h1")
        h2_d = dram.tile([R, H], F32, tag="h2")

        consts = top.enter_context(tc.tile_pool(name="consts", bufs=1))
        ident = consts.tile([128, 128], F32, tag="ident")
        make_identity(nc, ident[:])
        ones = consts.tile([1, 128], F32, tag="ones")
        nc.vector.memset(ones[:], 1.0)

        def load_chunks(ctx, name, src, n, width):
            pool = ctx.enter_context(tc.tile_pool(name=name, bufs=1))
            tiles = []
            for k in range(n):
                tl = pool.tile([128, width], F32, tag=f"c{k}")
                nc.sync.dma_start(out=tl[:], in_=src[128 * k:128 * (k + 1), :])
                tiles.append(tl)
            return tiles

        def load_row(ctx, name, src, width):
            pool = ctx.enter_context(tc.tile_pool(name=name, bufs=1))
            tl = pool.tile([1, width], F32)
            nc.sync.dma_start(out=tl[:], in_=src[0:1, :])
            return tl

        with ExitStack() as ctx:   # phase 1: xp1 = x @ W1 + b1
            w_sb = load_chunks(ctx, "w1s", w1[:], F // 128, G)
            b_sb = load_row(ctx, "b1s", b1[:], G)
            _xp_phase(nc, tc, ctx, xin[:], w_sb, b_sb, xp_d[:], ident, ones, F)
        with ExitStack() as ctx:   # phase 2: layer-1 scan
            u_sb = load_chunks(ctx, "u1s", u1[:], KC, G)
            _scan_phase(nc, tc, ctx, xp_d[:], u_sb, h1_d[:], ident, "l1")
        with ExitStack() as ctx:   # phase 3: xp2 = h1 @ W2 + b2
            w_sb = load_chunks(ctx, "w2s", w2[:], KC, G)
            b_sb = load_row(ctx, "b2s", b2[:], G)
            _xp_phase(nc, tc, ctx, h1_d[:], w_sb, b_sb, xp2_d[:], ident, ones, H)
        with ExitStack() as ctx:   # phase 4: layer-2 scan
            u_sb = load_chunks(ctx, "u2s", u2[:], KC, G)
            _scan_phase(nc, tc, ctx, xp2_d[:], u_sb, h2_d[:], ident, "l2")
        with ExitStack() as ctx:   # phase 5: folded heads
            wh_sb = load_chunks(ctx, "whs", wh[:], KC, OUT)
            bh_sb = load_row(ctx, "bhs", bh[:], OUT)
            _heads_phase(nc, tc, ctx, h2_d[:], wh_sb, bh_sb, out[:], ident, ones)

    nc.compile()
    return nc


_NC_CACHE = None


def _get_nc():
    global _NC_CACHE
    if _NC_CACHE is None:
        _NC_CACHE = _build()
    return _NC_CACHE


def kernel(x, W1, U1, b1, W2, U2, b2, Wh1, bh1, Wh2, bh2, Wh3, bh3, Wf, bf,
           _trace=False):
    x = np.asarray(x, dtype=np.float32)
    # fold the three heads + final linear into one affine map
    wh_cat = np.concatenate([np.asarray(Wh1), np.asarray(Wh2), np.asarray(Wh3)],
                            axis=1).astype(np.float32)
    bh_cat = np.concatenate([np.asarray(bh1), np.asarray(bh2), np.asarray(bh3)],
                            axis=0).astype(np.float32)
    wf = np.asarray(Wf, dtype=np.float32)
    wh_fold = np.ascontiguousarray(wh_cat @ wf)
    bh_fold = (bh_cat @ wf + np.asarray(bf, dtype=np.float32)).reshape(1, OUT)
    bh_fold = np.ascontiguousarray(bh_fold)

    shared = {
        "w1": _reorder_gates(np.asarray(W1, dtype=np.float32)),
        "u1": _reorder_gates(np.asarray(U1, dtype=np.float32)),
        "b1": _reorder_gates(np.asarray(b1, dtype=np.float32).reshape(1, G)),
        "w2": _reorder_gates(np.asarray(W2, dtype=np.float32)),
        "u2": _reorder_gates(np.asarray(U2, dtype=np.float32)),
        "b2": _reorder_gates(np.asarray(b2, dtype=np.float32).reshape(1, G)),
        "wh": wh_fold,
        "bh": bh_fold,
    }
    x_tb = np.swapaxes(x, 0, 1)   # [T, B, F], (t, b)-major
    in_maps = []
    for k in range(NCORES):
        xs = np.ascontiguousarray(x_tb[:, k * BS:(k + 1) * BS, :]).reshape(R, F)
        in_maps.append({"xin": xs, **shared})

    nc = _get_nc()
    res = run_bass_kernel_spmd(nc, in_maps, core_ids=list(range(NCORES)),
                               trace=_trace)
    outs = []
    for k in range(NCORES):
        o = res.results[k]["out"].reshape(T, BS, OUT)
        outs.append(np.swapaxes(o, 0, 1))   # back to [BS, T, OUT]
    full = np.concatenate(outs, axis=0)
    if _trace:
        return full, res
    return full

